# revision 11
# baseline (speedup 1.0000x reference)
"""Trainium2 Bass kernel for KMeans assignment (argmin over 8192 centroids).

Problem: x [32768, 1024] f32, centroids [1024, 8192] f32 ->
         argmin_k ||x_n - c_k||^2  as int32 [32768].

Math: argmin_k (||x||^2 - 2 x.c_k + ||c_k||^2) == argmax_k (x.c_k - 0.5*||c_k||^2).
The ||x||^2 term is row-constant and drops out of the argmin.

Sharding: data-parallel over N across 8 cores (4096 rows each), centroids
replicated.

Per core (engine assignment chosen to keep every engine under the fp8
tensor-engine floor of ~437 us):
 - PE (tensor): x and centroids pre-quantized to fp8 e4m3 on the host;
   DoubleRow fp8 matmuls (256-wide contraction per call, 2x throughput)
   accumulate 2048-wide PSUM tiles.
 - Activation: drains PSUM f32 -> f16 score tiles in SBUF (plain Copy;
   the DVE would be over budget if it also did this pass).
 - DVE: adds the f16 bias (-0.5*||c_k||^2) fused into a halving-max
   tree: 8192 -> 4096 -> 2048 -> 1024 via contiguous-halves
   tensor_tensor(max) (all-f16 operands hit the DVE 2x mode), then one
   max8 + max_index over the final 1024 slots.

Each of a row's top-8 slots expands to 8 raw centroid ids (j + 1024*m).
This grouping provably keeps every noisy-top-8 score's group in the
top-8 slots, so the true argmax (which sits in the noisy top few; fp8
quantization noise sigma~1.6 vs typical top-1/top-2 gap ~10) is always
among the 64 host candidates. The host re-scores candidates exactly in
fp64 and picks the argmax with first-index tie-breaking.
"""
import numpy as np

# ---- problem constants (hardcoded per harness contract) ----
N_FULL, D, K = 32768, 1024, 8192
N_CORES = 8
NC = N_FULL // N_CORES          # 4096 rows per core
T = NC // 128                   # 32 row-tiles per core
RB = 4                          # row-tiles per row block
NRB = T // RB                   # 8 row blocks
KQ = 4                          # K quarters
KQW = K // KQ                   # 2048 centroids per quarter
DC = 4                          # 256-wide contraction chunks (DoubleRow)
NSLOT = 1024                    # slots after 3 halvings
EXPAND = K // NSLOT             # raw candidates per slot

_compiled = {}


def _build(reps: int = 1, variant: str = "full"):
    """Build + compile the per-core Bass program (body unrolled `reps`
    times; reps>1 and variant!="full" exist only for timing diagnostics).

    variants: full | nodve (no DVE reduce) | noact (matmuls+DMA only) |
              mmonly (single cent load, no re-DMA) | dmaonly (DMAs only)
    """
    from contextlib import ExitStack
    import concourse.bacc as bacc
    import concourse.mybir as mybir
    import concourse.tile as tile

    f32 = mybir.dt.float32
    f16 = mybir.dt.float16
    f8 = mybir.dt.float8e4
    u16 = mybir.dt.uint16
    DR = mybir.MatmulPerfMode.DoubleRow
    ADD = mybir.AluOpType.add
    MAX = mybir.AluOpType.max

    do_mm = variant in ("full", "nodve", "noact", "mmonly")
    do_act = variant in ("full", "nodve")
    do_dve = variant in ("full",)
    do_cdma = variant in ("full", "nodve", "noact", "dmaonly")

    nc = bacc.Bacc("TRN2", target_bir_lowering=False, debug=False)

    xt_d = nc.dram_tensor("xt", [D, NC], f8, kind="ExternalInput").ap()
    c_d = nc.dram_tensor("cent", [D, K], f8, kind="ExternalInput").ap()
    bias_d = nc.dram_tensor("bias", [128, K], f16, kind="ExternalInput").ap()
    outv_d = nc.dram_tensor("outv", [128, T * 8], f16, kind="ExternalOutput").ap()
    outi_d = nc.dram_tensor("outi", [128, T * 8], u16, kind="ExternalOutput").ap()

    with tile.TileContext(nc) as tc:
        with ExitStack() as ctx:
            const_pool = ctx.enter_context(tc.tile_pool(name="const", bufs=1))
            xt_pool = ctx.enter_context(tc.tile_pool(name="xt", bufs=1))
            c_pool = ctx.enter_context(tc.tile_pool(name="cent", bufs=2))
            sc_pool = ctx.enter_context(tc.tile_pool(name="scores", bufs=1))
            tmp_pool = ctx.enter_context(tc.tile_pool(name="tmp", bufs=1))
            out_pool = ctx.enter_context(tc.tile_pool(name="out", bufs=1))
            ps_pool = ctx.enter_context(
                tc.tile_pool(name="psum", bufs=2, space="PSUM"))

            for _ in range(reps):
                bias_sb = const_pool.tile([128, K], f16, name="bias_sb",
                                          tag="bias")
                nc.sync.dma_start(bias_sb[:], bias_d[:])

                xt_sb = [xt_pool.tile([128, 2, NC], f8, name=f"xt_sb{dc}",
                                      tag=f"xt{dc}") for dc in range(DC)]
                for dc in range(DC):
                    for i in range(2):
                        r0 = dc * 256 + i * 128
                        nc.sync.dma_start(xt_sb[dc][:, i, :],
                                          xt_d[r0:r0 + 128, :])

                if do_dve:
                    mv_all = out_pool.tile([128, T * 8], f16, name="mv",
                                           tag="mv")
                    mi_all = out_pool.tile([128, T * 8], u16, name="mi",
                                           tag="mi")

                c_fixed = None
                if variant == "mmonly":
                    c_fixed = [c_pool.tile([128, 2, KQW], f8,
                                           name=f"c_sb{dc}", tag=f"c{dc}")
                               for dc in range(DC)]
                    for dc in range(DC):
                        for i in range(2):
                            r0 = dc * 256 + i * 128
                            nc.sync.dma_start(c_fixed[dc][:, i, :],
                                              c_d[r0:r0 + 128, 0:KQW])

                for rb in range(NRB):
                    sc_t = [sc_pool.tile([128, K], f16, name=f"sc{nt}",
                                         tag=f"sc{nt}") for nt in range(RB)] \
                        if do_act else None
                    for kq in range(KQ):
                        if do_cdma:
                            c_sb = [c_pool.tile([128, 2, KQW], f8,
                                                name=f"c_sb{dc}", tag=f"c{dc}")
                                    for dc in range(DC)]
                            for dc in range(DC):
                                for i in range(2):
                                    r0 = dc * 256 + i * 128
                                    nc.sync.dma_start(
                                        c_sb[dc][:, i, :],
                                        c_d[r0:r0 + 128,
                                            kq * KQW:(kq + 1) * KQW])
                        else:
                            c_sb = c_fixed
                        for nt in range(RB):
                            t = rb * RB + nt
                            if do_mm:
                                ps = ps_pool.tile([128, KQW], f32, name="ps",
                                                  tag="ps")
                                for dc in range(DC):
                                    for j in range(KQW // 512):
                                        nc.tensor.matmul(
                                            ps[:, j * 512:(j + 1) * 512],
                                            xt_sb[dc][:, :,
                                                      t * 128:(t + 1) * 128],
                                            c_sb[dc][:, :,
                                                     j * 512:(j + 1) * 512],
                                            start=(dc == 0),
                                            stop=(dc == DC - 1),
                                            perf_mode=DR)
                            if do_act:
                                nc.scalar.copy(
                                    sc_t[nt][:, kq * KQW:(kq + 1) * KQW],
                                    ps[:])
                    if do_dve:
                        for nt in range(RB):
                            t = rb * RB + nt
                            sc = sc_t[nt]
                            h = K // 2
                            tA = tmp_pool.tile([128, h], f16, name="tA",
                                               tag="tA")
                            tB = tmp_pool.tile([128, h], f16, name="tB",
                                               tag="tB")
                            tC = tmp_pool.tile([128, h], f16, name="tC",
                                               tag="tC")
                            tD = tmp_pool.tile([128, h // 2], f16, name="tD",
                                               tag="tD")
                            tE = tmp_pool.tile([128, h // 4], f16, name="tE",
                                               tag="tE")
                            nc.vector.tensor_tensor(tA[:], sc[:, 0:h],
                                                    bias_sb[:, 0:h], ADD)
                            nc.vector.tensor_tensor(tB[:], sc[:, h:K],
                                                    bias_sb[:, h:K], ADD)
                            nc.vector.tensor_tensor(tC[:], tA[:], tB[:], MAX)
                            nc.vector.tensor_tensor(tD[:], tC[:, 0:h // 2],
                                                    tC[:, h // 2:h], MAX)
                            nc.vector.tensor_tensor(tE[:], tD[:, 0:h // 4],
                                                    tD[:, h // 4:h // 2], MAX)
                            nc.vector.max(mv_all[:, t * 8:(t + 1) * 8], tE[:])
                            nc.vector.max_index(mi_all[:, t * 8:(t + 1) * 8],
                                                mv_all[:, t * 8:(t + 1) * 8],
                                                tE[:])

                if do_dve:
                    nc.sync.dma_start(outv_d[:], mv_all[:])
                    nc.sync.dma_start(outi_d[:], mi_all[:])
    nc.compile()
    return nc


def _get_nc(reps: int = 1, variant: str = "full"):
    key = (reps, variant)
    if key not in _compiled:
        _compiled[key] = _build(reps, variant)
    return _compiled[key]


def _prepare_in_maps(x: np.ndarray, centroids: np.ndarray):
    """Host-side prep shared by kernel() and the timing harness."""
    import ml_dtypes

    f8 = ml_dtypes.float8_e4m3
    xt8 = np.ascontiguousarray(x.T).astype(f8)               # [D, N]
    c8 = np.ascontiguousarray(centroids.astype(f8))          # [D, K]
    bias_row = -0.5 * np.einsum("dk,dk->k", centroids, centroids,
                                dtype=np.float64).astype(np.float16)
    bias = np.ascontiguousarray(np.broadcast_to(bias_row, (128, K)))
    in_maps = []
    for c in range(N_CORES):
        in_maps.append({
            "xt": np.ascontiguousarray(xt8[:, c * NC:(c + 1) * NC]),
            "cent": c8,
            "bias": bias,
        })
    return in_maps


def _candidates(outi: np.ndarray):
    """Device slot indices -> per-row raw candidate ids [NC, 8*EXPAND]."""
    slots = outi.reshape(128, T, 8).transpose(1, 0, 2).reshape(NC, 8)
    slots = np.clip(slots.astype(np.int64), 0, NSLOT - 1)
    offs = (np.arange(EXPAND, dtype=np.int64) * NSLOT)[None, None, :]
    return (slots[:, :, None] + offs).reshape(NC, 8 * EXPAND)


def _refine(xs, centroids, bias_row, cand):
    """Exact fp64 rescore of each row's candidates; argmax with
    first-index tie-break (matches reference argmin semantics)."""
    n = xs.shape[0]
    out = np.empty(n, dtype=np.int64)
    bs = 2048
    cT = np.ascontiguousarray(centroids.T)               # [K, D]
    for s in range(0, n, bs):
        e = min(s + bs, n)
        cb = cand[s:e]                                   # [b, m]
        cc = cT[cb]                                      # [b, m, D]
        sc = np.einsum("bd,bmd->bm", xs[s:e], cc, dtype=np.float64)
        sc = sc + bias_row[cb]
        best = sc.max(axis=1, keepdims=True)
        big = np.where(sc >= best, cb, np.iinfo(np.int64).max)
        out[s:e] = big.min(axis=1)
    return out.astype(np.int32)


def kernel(x: np.ndarray, centroids: np.ndarray) -> np.ndarray:
    from concourse.bass_utils import run_bass_kernel_spmd

    x = np.asarray(x, dtype=np.float32)
    centroids = np.asarray(centroids, dtype=np.float32)
    nc = _get_nc()
    in_maps = _prepare_in_maps(x, centroids)
    res = run_bass_kernel_spmd(nc, in_maps, core_ids=list(range(N_CORES)))

    bias_row = -0.5 * np.einsum("dk,dk->k", centroids, centroids,
                                dtype=np.float64)
    out = np.empty(N_FULL, dtype=np.int32)
    for c in range(N_CORES):
        cand = _candidates(res.results[c]["outi"])
        out[c * NC:(c + 1) * NC] = _refine(
            x[c * NC:(c + 1) * NC], centroids, bias_row, cand)
    return out


# revision 17
# speedup vs baseline: 2.0922x; 2.0922x over previous
"""Trainium2 Bass kernel for KMeans assignment (argmin over 8192 centroids).

Problem: x [32768, 1024] f32, centroids [1024, 8192] f32 ->
         argmin_k ||x_n - c_k||^2  as int32 [32768].

Math: argmin_k (||x||^2 - 2 x.c_k + ||c_k||^2) == argmax_k (x.c_k - 0.5*||c_k||^2).
The ||x||^2 term is row-constant and drops out of the argmin.

Sharding: data-parallel over N across 8 cores (4096 rows each), centroids
replicated.

Per core (engine assignment chosen to keep every engine under the fp8
tensor-engine floor of ~437 us):
 - PE (tensor): x and centroids pre-quantized to fp8 e4m3 on the host;
   DoubleRow fp8 matmuls (256-wide contraction per call, 2x throughput)
   accumulate 2048-wide PSUM tiles.
 - Activation: drains PSUM f32 -> f16 score tiles in SBUF (plain Copy;
   the DVE would be over budget if it also did this pass).
 - DVE: adds the f16 bias (-0.5*||c_k||^2) fused into a halving-max
   tree: 8192 -> 4096 -> 2048 -> 1024 via contiguous-halves
   tensor_tensor(max) (all-f16 operands hit the DVE 2x mode), then one
   max8 + max_index over the final 1024 slots.

Each of a row's top-8 slots expands to 8 raw centroid ids (j + 1024*m).
This grouping provably keeps every noisy-top-8 score's group in the
top-8 slots, so the true argmax (which sits in the noisy top few; fp8
quantization noise sigma~1.6 vs typical top-1/top-2 gap ~10) is always
among the 64 host candidates. The host re-scores candidates exactly in
fp64 and picks the argmax with first-index tie-breaking.
"""
import numpy as np

# ---- problem constants (hardcoded per harness contract) ----
N_FULL, D, K = 32768, 1024, 8192
N_CORES = 8
NC = N_FULL // N_CORES          # 4096 rows per core
T = NC // 128                   # 32 row-tiles per core
RB = 4                          # row-tiles per row block
NRB = T // RB                   # 8 row blocks
KQ = 4                          # K quarters
KQW = K // KQ                   # 2048 centroids per quarter
DC = 4                          # 256-wide contraction chunks (DoubleRow)
NSLOT = 1024                    # slots after 3 halvings
EXPAND = K // NSLOT             # raw candidates per slot

_compiled = {}


def _build(reps: int = 1, variant: str = "full", fake: bool = False):
    """Build + compile the per-core Bass program (body unrolled `reps`
    times; reps>1, variant!="full", and fake=True exist only for timing
    diagnostics — fake replaces the big ExternalInputs with Internal DRAM
    scratch (garbage values; timing is value-independent) so timing
    dispatches ship no input bytes over the axon tunnel).

    variants: full | nodve (no DVE reduce) | noact (matmuls+DMA only) |
              mmonly (single cent load, no re-DMA) | dmaonly (DMAs only)
    """
    from contextlib import ExitStack
    import concourse.bacc as bacc
    import concourse.mybir as mybir
    import concourse.tile as tile

    f32 = mybir.dt.float32
    f16 = mybir.dt.float16
    f8 = mybir.dt.float8e4
    u16 = mybir.dt.uint16
    DR = mybir.MatmulPerfMode.DoubleRow
    ADD = mybir.AluOpType.add
    MAX = mybir.AluOpType.max

    do_mm = variant in ("full", "nodve", "noact", "mmonly")
    do_act = variant in ("full", "nodve")
    do_dve = variant in ("full",)
    do_cdma = variant in ("full", "nodve", "noact", "dmaonly")

    nc = bacc.Bacc("TRN2", target_bir_lowering=False, debug=False)

    in_kind = "Internal" if fake else "ExternalInput"
    xt_d = nc.dram_tensor("xt", [D, NC], f8, kind=in_kind).ap()
    c_d = nc.dram_tensor("cent", [D, K], f8, kind=in_kind).ap()
    bias_d = nc.dram_tensor("bias", [128, K], f16, kind=in_kind).ap()
    outv_d = nc.dram_tensor("outv", [128, T * 8], f16, kind="ExternalOutput").ap()
    outi_d = nc.dram_tensor("outi", [128, T * 8], u16, kind="ExternalOutput").ap()

    import contextlib

    with tile.TileContext(nc) as tc:
        with ExitStack() as ctx:
            const_pool = ctx.enter_context(tc.tile_pool(name="const", bufs=1))
            xt_pool = ctx.enter_context(tc.tile_pool(name="xt", bufs=1))
            c_pool = ctx.enter_context(tc.tile_pool(name="cent", bufs=2))
            sc_pool = ctx.enter_context(tc.tile_pool(name="scores", bufs=1))
            tmp_pool = ctx.enter_context(tc.tile_pool(name="tmp", bufs=1))
            out_pool = ctx.enter_context(tc.tile_pool(name="out", bufs=1))
            ps_pool = ctx.enter_context(
                tc.tile_pool(name="psum", bufs=2, space="PSUM"))

            loop_ctx = (tc.For_i(0, reps) if reps > 1
                        else contextlib.nullcontext())
            with loop_ctx:
                xt_sb = [xt_pool.tile([128, 2, NC], f8, name=f"xt_sb{dc}",
                                      tag=f"xt{dc}") for dc in range(DC)]
                for dc in range(DC):
                    for i in range(2):
                        r0 = dc * 256 + i * 128
                        nc.sync.dma_start(xt_sb[dc][:, i, :],
                                          xt_d[r0:r0 + 128, :])

                # bias is consumed late (DVE reduce); issue its DMA after
                # xt so it never delays the first matmuls
                bias_sb = const_pool.tile([128, K], f16, name="bias_sb",
                                          tag="bias")
                nc.sync.dma_start(bias_sb[:], bias_d[:])

                if do_dve:
                    mv_all = out_pool.tile([128, T * 8], f16, name="mv",
                                           tag="mv")
                    mi_all = out_pool.tile([128, T * 8], u16, name="mi",
                                           tag="mi")

                c_fixed = None
                if variant == "mmonly":
                    c_fixed = [c_pool.tile([128, 2, KQW], f8,
                                           name=f"c_sb{dc}", tag=f"c{dc}")
                               for dc in range(DC)]
                    for dc in range(DC):
                        for i in range(2):
                            r0 = dc * 256 + i * 128
                            nc.sync.dma_start(c_fixed[dc][:, i, :],
                                              c_d[r0:r0 + 128, 0:KQW])

                for rb in range(NRB):
                    sc_t = [sc_pool.tile([128, K], f16, name=f"sc{nt}",
                                         tag=f"sc{nt}") for nt in range(RB)] \
                        if do_act else None
                    for kq in range(KQ):
                        if do_cdma:
                            c_sb = [c_pool.tile([128, 2, KQW], f8,
                                                name=f"c_sb{dc}", tag=f"c{dc}")
                                    for dc in range(DC)]
                            for dc in range(DC):
                                for i in range(2):
                                    r0 = dc * 256 + i * 128
                                    nc.sync.dma_start(
                                        c_sb[dc][:, i, :],
                                        c_d[r0:r0 + 128,
                                            kq * KQW:(kq + 1) * KQW])
                        else:
                            c_sb = c_fixed
                        for nt in range(RB):
                            t = rb * RB + nt
                            if do_mm:
                                ps = ps_pool.tile([128, KQW], f32, name="ps",
                                                  tag="ps")
                                for dc in range(DC):
                                    for j in range(KQW // 512):
                                        nc.tensor.matmul(
                                            ps[:, j * 512:(j + 1) * 512],
                                            xt_sb[dc][:, :,
                                                      t * 128:(t + 1) * 128],
                                            c_sb[dc][:, :,
                                                     j * 512:(j + 1) * 512],
                                            start=(dc == 0),
                                            stop=(dc == DC - 1),
                                            perf_mode=DR)
                            if do_act:
                                nc.scalar.copy(
                                    sc_t[nt][:, kq * KQW:(kq + 1) * KQW],
                                    ps[:])
                    if do_dve:
                        for nt in range(RB):
                            t = rb * RB + nt
                            sc = sc_t[nt]
                            h = K // 2
                            tA = tmp_pool.tile([128, h], f16, name="tA",
                                               tag="tA")
                            tB = tmp_pool.tile([128, h], f16, name="tB",
                                               tag="tB")
                            tC = tmp_pool.tile([128, h], f16, name="tC",
                                               tag="tC")
                            tD = tmp_pool.tile([128, h // 2], f16, name="tD",
                                               tag="tD")
                            tE = tmp_pool.tile([128, h // 4], f16, name="tE",
                                               tag="tE")
                            nc.vector.tensor_tensor(tA[:], sc[:, 0:h],
                                                    bias_sb[:, 0:h], ADD)
                            nc.vector.tensor_tensor(tB[:], sc[:, h:K],
                                                    bias_sb[:, h:K], ADD)
                            nc.vector.tensor_tensor(tC[:], tA[:], tB[:], MAX)
                            nc.vector.tensor_tensor(tD[:], tC[:, 0:h // 2],
                                                    tC[:, h // 2:h], MAX)
                            nc.vector.tensor_tensor(tE[:], tD[:, 0:h // 4],
                                                    tD[:, h // 4:h // 2], MAX)
                            nc.vector.max(mv_all[:, t * 8:(t + 1) * 8], tE[:])
                            nc.vector.max_index(mi_all[:, t * 8:(t + 1) * 8],
                                                mv_all[:, t * 8:(t + 1) * 8],
                                                tE[:])

                if do_dve:
                    nc.sync.dma_start(outv_d[:], mv_all[:])
                    nc.sync.dma_start(outi_d[:], mi_all[:])
    nc.compile()
    return nc


def _get_nc(reps: int = 1, variant: str = "full", fake: bool = False):
    key = (reps, variant, fake)
    if key not in _compiled:
        _compiled[key] = _build(reps, variant, fake)
    return _compiled[key]


def _prepare_in_maps(x: np.ndarray, centroids: np.ndarray):
    """Host-side prep shared by kernel() and the timing harness."""
    import ml_dtypes

    f8 = ml_dtypes.float8_e4m3
    xt8 = np.ascontiguousarray(x.T).astype(f8)               # [D, N]
    c8 = np.ascontiguousarray(centroids.astype(f8))          # [D, K]
    bias_row = -0.5 * np.einsum("dk,dk->k", centroids, centroids,
                                dtype=np.float64).astype(np.float16)
    bias = np.ascontiguousarray(np.broadcast_to(bias_row, (128, K)))
    in_maps = []
    for c in range(N_CORES):
        in_maps.append({
            "xt": np.ascontiguousarray(xt8[:, c * NC:(c + 1) * NC]),
            "cent": c8,
            "bias": bias,
        })
    return in_maps


def _candidates(outi: np.ndarray):
    """Device slot indices -> per-row raw candidate ids [NC, 8*EXPAND]."""
    slots = outi.reshape(128, T, 8).transpose(1, 0, 2).reshape(NC, 8)
    slots = np.clip(slots.astype(np.int64), 0, NSLOT - 1)
    offs = (np.arange(EXPAND, dtype=np.int64) * NSLOT)[None, None, :]
    return (slots[:, :, None] + offs).reshape(NC, 8 * EXPAND)


def _refine(xs, centroids, bias_row, cand):
    """Exact fp64 rescore of each row's candidates; argmax with
    first-index tie-break (matches reference argmin semantics)."""
    n = xs.shape[0]
    out = np.empty(n, dtype=np.int64)
    bs = 2048
    cT = np.ascontiguousarray(centroids.T)               # [K, D]
    for s in range(0, n, bs):
        e = min(s + bs, n)
        cb = cand[s:e]                                   # [b, m]
        cc = cT[cb]                                      # [b, m, D]
        sc = np.einsum("bd,bmd->bm", xs[s:e], cc, dtype=np.float64)
        sc = sc + bias_row[cb]
        best = sc.max(axis=1, keepdims=True)
        big = np.where(sc >= best, cb, np.iinfo(np.int64).max)
        out[s:e] = big.min(axis=1)
    return out.astype(np.int32)


def kernel(x: np.ndarray, centroids: np.ndarray) -> np.ndarray:
    from concourse.bass_utils import run_bass_kernel_spmd

    x = np.asarray(x, dtype=np.float32)
    centroids = np.asarray(centroids, dtype=np.float32)
    nc = _get_nc()
    in_maps = _prepare_in_maps(x, centroids)
    res = run_bass_kernel_spmd(nc, in_maps, core_ids=list(range(N_CORES)))

    bias_row = -0.5 * np.einsum("dk,dk->k", centroids, centroids,
                                dtype=np.float64)
    out = np.empty(N_FULL, dtype=np.int32)
    for c in range(N_CORES):
        cand = _candidates(res.results[c]["outi"])
        out[c * NC:(c + 1) * NC] = _refine(
            x[c * NC:(c + 1) * NC], centroids, bias_row, cand)
    return out


# revision 25
# speedup vs baseline: 2.6560x; 1.2694x over previous
"""Trainium2 Bass kernel for KMeans assignment (argmin over 8192 centroids).

Problem: x [32768, 1024] f32, centroids [1024, 8192] f32 ->
         argmin_k ||x_n - c_k||^2  as int32 [32768].

Math: argmin_k (||x||^2 - 2 x.c_k + ||c_k||^2) == argmax_k (x.c_k - 0.5*||c_k||^2).
The ||x||^2 term is row-constant and drops out of the argmin.

Sharding: data-parallel over N across 8 cores (4096 rows each), centroids
replicated.

Per core (engine assignment chosen to keep every engine under the fp8
tensor-engine floor of ~437 us):
 - PE (tensor): x and centroids pre-quantized to fp8 e4m3 on the host;
   DoubleRow fp8 matmuls (256-wide contraction per call, 2x throughput)
   accumulate 2048-wide PSUM tiles.
 - Activation: drains PSUM f32 -> f16 score tiles in SBUF (plain Copy;
   the DVE would be over budget if it also did this pass).
 - DVE: adds the f16 bias (-0.5*||c_k||^2) fused into a halving-max
   tree: 8192 -> 4096 -> 2048 -> 1024 via contiguous-halves
   tensor_tensor(max) (all-f16 operands hit the DVE 2x mode), then one
   max8 + max_index over the final 1024 slots.

Each of a row's top-8 slots expands to 8 raw centroid ids (j + 1024*m).
This grouping provably keeps every noisy-top-8 score's group in the
top-8 slots, so the true argmax (which sits in the noisy top few; fp8
quantization noise sigma~1.6 vs typical top-1/top-2 gap ~10) is always
among the 64 host candidates. The host re-scores candidates exactly in
fp64 and picks the argmax with first-index tie-breaking.
"""
import numpy as np

# ---- problem constants (hardcoded per harness contract) ----
N_FULL, D, K = 32768, 1024, 8192
N_CORES = 8
NC = N_FULL // N_CORES          # 4096 rows per core
T = NC // 128                   # 32 row-tiles per core
RB = 4                          # row-tiles per row block
NRB = T // RB                   # 8 row blocks
KQ = 4                          # K quarters
KQW = K // KQ                   # 2048 centroids per quarter
DC = 4                          # 256-wide contraction chunks (DoubleRow)
NSLOT = 1024                    # slots after 3 halvings
EXPAND = K // NSLOT             # raw candidates per slot
import os as _os
JW = int(_os.environ.get("KMEANS_JW", "512"))   # matmul output width
WLOAD = int(_os.environ.get("KMEANS_WLOAD", "1"))  # K-half weight-reuse layout

_compiled = {}


def _build(reps: int = 1, variant: str = "full", fake: bool = False):
    """Build + compile the per-core Bass program (body unrolled `reps`
    times; reps>1, variant!="full", and fake=True exist only for timing
    diagnostics — fake replaces the big ExternalInputs with Internal DRAM
    scratch (garbage values; timing is value-independent) so timing
    dispatches ship no input bytes over the axon tunnel).

    variants: full | nodve (no DVE reduce) | noact (matmuls+DMA only) |
              mmonly (single cent load, no re-DMA) | dmaonly (DMAs only)
    """
    from contextlib import ExitStack
    import concourse.bacc as bacc
    import concourse.mybir as mybir
    import concourse.tile as tile

    f32 = mybir.dt.float32
    f16 = mybir.dt.float16
    f8 = mybir.dt.float8e4
    u16 = mybir.dt.uint16
    DR = mybir.MatmulPerfMode.DoubleRow
    ADD = mybir.AluOpType.add
    MAX = mybir.AluOpType.max

    do_mm = variant in ("full", "nodve", "noact", "mmonly")
    do_act = variant in ("full", "nodve")
    do_dve = variant in ("full",)
    do_cdma = variant in ("full", "nodve", "noact", "dmaonly")

    nc = bacc.Bacc("TRN2", target_bir_lowering=False, debug=False)

    in_kind = "Internal" if fake else "ExternalInput"
    xt_d = nc.dram_tensor("xt", [D, NC], f8, kind=in_kind).ap()
    c_d = nc.dram_tensor("cent", [D, K], f8, kind=in_kind).ap()
    bias_d = nc.dram_tensor("bias", [128, K], f16, kind=in_kind).ap()
    outv_d = nc.dram_tensor("outv", [128, T * 8], f16, kind="ExternalOutput").ap()
    outi_d = nc.dram_tensor("outi", [128, T * 8], u16, kind="ExternalOutput").ap()

    import contextlib

    with tile.TileContext(nc) as tc:
        with ExitStack() as ctx:
            const_pool = ctx.enter_context(tc.tile_pool(name="const", bufs=1))
            xt_pool = ctx.enter_context(tc.tile_pool(name="xt", bufs=1))
            c_pool = ctx.enter_context(tc.tile_pool(name="cent", bufs=2))
            sc_pool = ctx.enter_context(tc.tile_pool(name="scores", bufs=1))
            tmp_pool = ctx.enter_context(tc.tile_pool(name="tmp", bufs=1))
            out_pool = ctx.enter_context(tc.tile_pool(name="out", bufs=1))
            ps_pool = ctx.enter_context(
                tc.tile_pool(name="psum", bufs=(1 if WLOAD else 2),
                             space="PSUM"))

            loop_ctx = (tc.For_i(0, reps) if reps > 1
                        else contextlib.nullcontext())
            with loop_ctx:
                xt_sb = [xt_pool.tile([128, 2, NC], f8, name=f"xt_sb{dc}",
                                      tag=f"xt{dc}") for dc in range(DC)]
                for dc in range(DC):
                    for i in range(2):
                        r0 = dc * 256 + i * 128
                        nc.sync.dma_start(xt_sb[dc][:, i, :],
                                          xt_d[r0:r0 + 128, :])

                # bias is consumed late (DVE reduce); issue its DMA after
                # xt so it never delays the first matmuls
                bias_sb = const_pool.tile([128, K], f16, name="bias_sb",
                                          tag="bias")
                nc.sync.dma_start(bias_sb[:], bias_d[:])

                if do_dve:
                    mv_all = out_pool.tile([128, T * 8], f16, name="mv",
                                           tag="mv")
                    mi_all = out_pool.tile([128, T * 8], u16, name="mi",
                                           tag="mi")

                c_fixed = None
                if variant == "mmonly":
                    cw = (K // 2) if WLOAD else KQW
                    c_fixed = [c_pool.tile([128, 2, cw], f8,
                                           name=f"c_sb{dc}", tag=f"c{dc}")
                               for dc in range(DC)]
                    for dc in range(DC):
                        for i in range(2):
                            r0 = dc * 256 + i * 128
                            nc.sync.dma_start(c_fixed[dc][:, i, :],
                                              c_d[r0:r0 + 128, 0:cw])

                for rb in range(NRB):
                    sc_t = [sc_pool.tile([128, K], f16, name=f"sc{nt}",
                                         tag=f"sc{nt}") for nt in range(RB)] \
                        if do_act else None
                    if WLOAD:
                        # K-half sweep: one lhsT load feeds 8 consecutive
                        # matmuls across 8 single-buffered PSUM banks.
                        KH = K // 2
                        for half in range(2):
                            if do_cdma:
                                c_sb = [c_pool.tile([128, 2, KH], f8,
                                                    name=f"c_sb{dc}",
                                                    tag=f"c{dc}")
                                        for dc in range(DC)]
                                for dc in range(DC):
                                    for i in range(2):
                                        r0 = dc * 256 + i * 128
                                        nc.sync.dma_start(
                                            c_sb[dc][:, i, :],
                                            c_d[r0:r0 + 128,
                                                half * KH:(half + 1) * KH])
                            else:
                                c_sb = c_fixed
                            for nt in range(RB):
                                t = rb * RB + nt
                                if do_mm:
                                    ps4 = [ps_pool.tile([128, 1024], f32,
                                                        name=f"ps{q}",
                                                        tag=f"ps{q}")
                                           for q in range(KH // 1024)]
                                    for dc in range(DC):
                                        for q in range(KH // 512):
                                            nc.tensor.matmul(
                                                ps4[q // 2][:, (q % 2) * 512:
                                                            (q % 2 + 1) * 512],
                                                xt_sb[dc][:, :,
                                                          t * 128:(t + 1) * 128],
                                                c_sb[dc][:, :,
                                                         q * 512:(q + 1) * 512],
                                                start=(dc == 0),
                                                stop=(dc == DC - 1),
                                                perf_mode=DR)
                                if do_act:
                                    for q in range(KH // 1024):
                                        nc.scalar.copy(
                                            sc_t[nt][:, half * KH + q * 1024:
                                                     half * KH + (q + 1) * 1024],
                                            ps4[q])
                        kq_iter = []
                    else:
                        kq_iter = range(KQ)
                    for kq in kq_iter:
                        if do_cdma:
                            c_sb = [c_pool.tile([128, 2, KQW], f8,
                                                name=f"c_sb{dc}", tag=f"c{dc}")
                                    for dc in range(DC)]
                            for dc in range(DC):
                                for i in range(2):
                                    r0 = dc * 256 + i * 128
                                    nc.sync.dma_start(
                                        c_sb[dc][:, i, :],
                                        c_d[r0:r0 + 128,
                                            kq * KQW:(kq + 1) * KQW])
                        else:
                            c_sb = c_fixed
                        for nt in range(RB):
                            t = rb * RB + nt
                            if do_mm:
                                ps = ps_pool.tile([128, KQW], f32, name="ps",
                                                  tag="ps")
                                for dc in range(DC):
                                    for j in range(KQW // JW):
                                        nc.tensor.matmul(
                                            ps[:, j * JW:(j + 1) * JW],
                                            xt_sb[dc][:, :,
                                                      t * 128:(t + 1) * 128],
                                            c_sb[dc][:, :,
                                                     j * JW:(j + 1) * JW],
                                            start=(dc == 0),
                                            stop=(dc == DC - 1),
                                            perf_mode=DR)
                            if do_act:
                                nc.scalar.copy(
                                    sc_t[nt][:, kq * KQW:(kq + 1) * KQW],
                                    ps[:])
                    if do_dve:
                        for nt in range(RB):
                            t = rb * RB + nt
                            sc = sc_t[nt]
                            h = K // 2
                            tA = tmp_pool.tile([128, h], f16, name="tA",
                                               tag="tA")
                            tB = tmp_pool.tile([128, h], f16, name="tB",
                                               tag="tB")
                            tC = tmp_pool.tile([128, h], f16, name="tC",
                                               tag="tC")
                            tD = tmp_pool.tile([128, h // 2], f16, name="tD",
                                               tag="tD")
                            tE = tmp_pool.tile([128, h // 4], f16, name="tE",
                                               tag="tE")
                            nc.vector.tensor_tensor(tA[:], sc[:, 0:h],
                                                    bias_sb[:, 0:h], ADD)
                            nc.vector.tensor_tensor(tB[:], sc[:, h:K],
                                                    bias_sb[:, h:K], ADD)
                            nc.vector.tensor_tensor(tC[:], tA[:], tB[:], MAX)
                            nc.vector.tensor_tensor(tD[:], tC[:, 0:h // 2],
                                                    tC[:, h // 2:h], MAX)
                            nc.vector.tensor_tensor(tE[:], tD[:, 0:h // 4],
                                                    tD[:, h // 4:h // 2], MAX)
                            nc.vector.max(mv_all[:, t * 8:(t + 1) * 8], tE[:])
                            nc.vector.max_index(mi_all[:, t * 8:(t + 1) * 8],
                                                mv_all[:, t * 8:(t + 1) * 8],
                                                tE[:])

                if do_dve:
                    nc.sync.dma_start(outv_d[:], mv_all[:])
                    nc.sync.dma_start(outi_d[:], mi_all[:])
    nc.compile()
    return nc


def _get_nc(reps: int = 1, variant: str = "full", fake: bool = False):
    key = (reps, variant, fake)
    if key not in _compiled:
        _compiled[key] = _build(reps, variant, fake)
    return _compiled[key]


def _prepare_in_maps(x: np.ndarray, centroids: np.ndarray):
    """Host-side prep shared by kernel() and the timing harness."""
    import ml_dtypes

    f8 = ml_dtypes.float8_e4m3
    xt8 = np.ascontiguousarray(x.T).astype(f8)               # [D, N]
    c8 = np.ascontiguousarray(centroids.astype(f8))          # [D, K]
    bias_row = -0.5 * np.einsum("dk,dk->k", centroids, centroids,
                                dtype=np.float64).astype(np.float16)
    bias = np.ascontiguousarray(np.broadcast_to(bias_row, (128, K)))
    in_maps = []
    for c in range(N_CORES):
        in_maps.append({
            "xt": np.ascontiguousarray(xt8[:, c * NC:(c + 1) * NC]),
            "cent": c8,
            "bias": bias,
        })
    return in_maps


def _candidates(outi: np.ndarray):
    """Device slot indices -> per-row raw candidate ids [NC, 8*EXPAND]."""
    slots = outi.reshape(128, T, 8).transpose(1, 0, 2).reshape(NC, 8)
    slots = np.clip(slots.astype(np.int64), 0, NSLOT - 1)
    offs = (np.arange(EXPAND, dtype=np.int64) * NSLOT)[None, None, :]
    return (slots[:, :, None] + offs).reshape(NC, 8 * EXPAND)


def _refine(xs, centroids, bias_row, cand):
    """Exact fp64 rescore of each row's candidates; argmax with
    first-index tie-break (matches reference argmin semantics)."""
    n = xs.shape[0]
    out = np.empty(n, dtype=np.int64)
    bs = 2048
    cT = np.ascontiguousarray(centroids.T)               # [K, D]
    for s in range(0, n, bs):
        e = min(s + bs, n)
        cb = cand[s:e]                                   # [b, m]
        cc = cT[cb]                                      # [b, m, D]
        sc = np.einsum("bd,bmd->bm", xs[s:e], cc, dtype=np.float64)
        sc = sc + bias_row[cb]
        best = sc.max(axis=1, keepdims=True)
        big = np.where(sc >= best, cb, np.iinfo(np.int64).max)
        out[s:e] = big.min(axis=1)
    return out.astype(np.int32)


def kernel(x: np.ndarray, centroids: np.ndarray) -> np.ndarray:
    from concourse.bass_utils import run_bass_kernel_spmd

    x = np.asarray(x, dtype=np.float32)
    centroids = np.asarray(centroids, dtype=np.float32)
    nc = _get_nc()
    in_maps = _prepare_in_maps(x, centroids)
    res = run_bass_kernel_spmd(nc, in_maps, core_ids=list(range(N_CORES)))

    bias_row = -0.5 * np.einsum("dk,dk->k", centroids, centroids,
                                dtype=np.float64)
    out = np.empty(N_FULL, dtype=np.int32)
    for c in range(N_CORES):
        cand = _candidates(res.results[c]["outi"])
        out[c * NC:(c + 1) * NC] = _refine(
            x[c * NC:(c + 1) * NC], centroids, bias_row, cand)
    return out
